# revision 1
# baseline (speedup 1.0000x reference)
"""Trainium2 Bass kernel for nn_ConfidenceAdaptiveSystem (MoE confidence routing).

Reference semantics (B=8192, D=4096, H=8192, C=2):
    t_out = relu(x @ t_w1 + t_b1) @ t_w2 + t_b2
    conf  = max(softmax(t_out, axis=1))          # == sigmoid(|t0 - t1|) for C=2
    f_out = relu(x @ f_w1 + f_b1) @ f_w2 + f_b2
    out   = where(conf < 0.8, f_out, t_out)

Sharding: data-parallel over the batch dim across 8 cores (1024 rows each);
weights replicated. Everything is computed transposed ([feature, batch]) so
the PE's stationary operand is always a weight tile and biases land on the
partition dim.

Precision strategy (from CPU error simulation):
  - The routing decision conf<0.8 <=> |t0-t1| < ln(4) is razor-thin: one
    flipped row costs ~1.6e-2 global relative error, so the t-expert path
    must be near-fp32. It uses a 3-pass bf16 split matmul
    (x_hi@w_hi + x_hi@w_lo + x_lo@w_hi, fp32 PSUM accumulate, ~2^-18
    coverage) and an fp32 second-layer matmul.
  - The f-expert only contributes output *values* (never the routing
    decision), so a single bf16 pass (~1.5e-3 L2 error) suffices.
This gives 4 full-size matmul passes instead of 8 (plain fp32 = 4 cyc/row
on the PE vs 1 cyc/row for bf16).
"""

import numpy as np
import ml_dtypes

import concourse.bass as bass
import concourse.mybir as mybir
from concourse.tile import TileContext
from concourse.bass_utils import run_bass_kernel_spmd

F32 = mybir.dt.float32
BF16 = mybir.dt.bfloat16
LN4 = float(np.log(0.8 / 0.2))  # |t0-t1| < LN4  <=>  conf < 0.8

N_CORES = 8
B, D, H, C = 8192, 4096, 8192, 2


def build_nc(Bc, D_, H_):
    """Build the per-core Bass program. Bc: rows per core."""
    KT = D_ // 128   # k-tiles of matmul1
    MT = H_ // 128   # m-tiles (h dim)
    NT = Bc // 512   # n-tiles (batch dim)
    nc = bass.Bass(trn_type="TRN2")

    # ---- DRAM parameters (per core) ----
    xhi = nc.declare_dram_parameter("xhi", [D_, Bc], BF16, isOutput=False)
    xlo = nc.declare_dram_parameter("xlo", [D_, Bc], BF16, isOutput=False)
    # weight slabs pre-arranged on host: [MT, 128, KT*128]
    # w_pre[m, p, k*128 + c] = w1[k*128 + p, m*128 + c]
    twh = nc.declare_dram_parameter("twh", [MT, 128, KT, 128], BF16, isOutput=False)
    twl = nc.declare_dram_parameter("twl", [MT, 128, KT, 128], BF16, isOutput=False)
    fwh = nc.declare_dram_parameter("fwh", [MT, 128, KT, 128], BF16, isOutput=False)
    # biases b1: [128, MT] with b1s[p, m] = b1[m*128 + p]
    tb1 = nc.declare_dram_parameter("tb1", [128, MT], F32, isOutput=False)
    fb1 = nc.declare_dram_parameter("fb1", [128, MT], F32, isOutput=False)
    # w2: [128, MT*2] with w2s[p, 2m:2m+2] = w2[m*128+p, :]
    tw2 = nc.declare_dram_parameter("tw2", [128, MT * 2], F32, isOutput=False)
    fw2 = nc.declare_dram_parameter("fw2", [128, MT * 2], BF16, isOutput=False)
    tb2 = nc.declare_dram_parameter("tb2", [2, 1], F32, isOutput=False)
    fb2 = nc.declare_dram_parameter("fb2", [2, 1], F32, isOutput=False)
    # [1,-1] and [1,1] helper vectors for the routing epilogue
    dvec = nc.declare_dram_parameter("dvec", [2, 1], F32, isOutput=False)
    ones12 = nc.declare_dram_parameter("ones12", [1, 2], F32, isOutput=False)
    out = nc.declare_dram_parameter("out", [2, Bc], F32, isOutput=True)

    with TileContext(nc) as tc:
        with (
            tc.tile_pool(name="xres", bufs=1) as xpool,
            tc.tile_pool(name="consts", bufs=1) as cpool,
            tc.tile_pool(name="wstream", bufs=3) as wpool,
            tc.tile_pool(name="hbuf", bufs=3) as hpool,
            tc.tile_pool(name="epi", bufs=1) as epool,
            tc.tile_pool(name="psmm", bufs=4, space="PSUM") as pspool,
            tc.tile_pool(name="pslg", bufs=2, space="PSUM") as ps2pool,
        ):
            # ---- resident loads ----
            xh_t = []
            xl_t = []
            for k in range(KT):
                th = xpool.tile([128, Bc], BF16, name=f"xh{k}")
                nc.sync.dma_start(out=th[:], in_=xhi[k * 128:(k + 1) * 128, :])
                xh_t.append(th)
                tl = xpool.tile([128, Bc], BF16, name=f"xl{k}")
                nc.sync.dma_start(out=tl[:], in_=xlo[k * 128:(k + 1) * 128, :])
                xl_t.append(tl)
            tb1_sb = cpool.tile([128, MT], F32, name="tb1sb")
            nc.sync.dma_start(out=tb1_sb[:], in_=tb1[:])
            fb1_sb = cpool.tile([128, MT], F32, name="fb1sb")
            nc.sync.dma_start(out=fb1_sb[:], in_=fb1[:])
            tw2_sb = cpool.tile([128, MT * 2], F32, name="tw2sb")
            nc.sync.dma_start(out=tw2_sb[:], in_=tw2[:])
            fw2_sb = cpool.tile([128, MT * 2], BF16, name="fw2sb")
            nc.sync.dma_start(out=fw2_sb[:], in_=fw2[:])
            tb2_sb = cpool.tile([2, 1], F32, name="tb2sb")
            nc.sync.dma_start(out=tb2_sb[:], in_=tb2[:])
            fb2_sb = cpool.tile([2, 1], F32, name="fb2sb")
            nc.sync.dma_start(out=fb2_sb[:], in_=fb2[:])
            dvec_sb = cpool.tile([2, 1], F32, name="dvecsb")
            nc.sync.dma_start(out=dvec_sb[:], in_=dvec[:])
            ones_sb = cpool.tile([1, 2], F32, name="onessb")
            nc.sync.dma_start(out=ones_sb[:], in_=ones12[:])

            logits = {}  # (expert, n) -> sbuf [2, 512] fp32

            # ---------------- t expert: 3-pass bf16 ----------------
            ps2_t = [ps2pool.tile([2, 512], F32, name=f"ps2t{n}", tag="ps2")
                     for n in range(NT)]
            for m in range(MT):
                wh = wpool.tile([128, KT, 128], BF16, name="wh", tag="wh")
                nc.sync.dma_start(out=wh[:], in_=twh[m])
                wl = wpool.tile([128, KT, 128], BF16, name="wl", tag="wl")
                nc.sync.dma_start(out=wl[:], in_=twl[m])
                pss = [pspool.tile([128, 512], F32, name=f"ps{n}", tag="ps")
                       for n in range(NT)]
                n_mm = KT * 3
                i_mm = 0
                for k in range(KT):
                    for (wt, xt) in ((wh, xh_t[k]), (wl, xh_t[k]), (wh, xl_t[k])):
                        for n in range(NT):
                            nc.tensor.matmul(
                                pss[n][:],
                                wt[:, k],
                                xt[:, n * 512:(n + 1) * 512],
                                start=(i_mm == 0),
                                stop=(i_mm == n_mm - 1),
                            )
                        i_mm += 1
                for n in range(NT):
                    ht = hpool.tile([128, 512], F32, name="ht", tag="ht")
                    nc.scalar.activation(
                        ht[:], pss[n][:], mybir.ActivationFunctionType.Relu,
                        bias=tb1_sb[:, m:m + 1],
                    )
                    nc.tensor.matmul(
                        ps2_t[n][:],
                        tw2_sb[:, 2 * m:2 * m + 2],
                        ht[:],
                        start=(m == 0),
                        stop=(m == MT - 1),
                    )
            for n in range(NT):
                lg = epool.tile([2, 512], F32, name=f"tlg{n}")
                nc.vector.tensor_scalar_add(lg[:], ps2_t[n][:], tb2_sb[:, 0:1])
                logits[("t", n)] = lg

            # ---------------- f expert: 1-pass bf16 ----------------
            ps2_f = [ps2pool.tile([2, 512], F32, name=f"ps2f{n}", tag="ps2")
                     for n in range(NT)]
            for m in range(MT):
                wh = wpool.tile([128, KT, 128], BF16, name="wh", tag="wh")
                nc.sync.dma_start(out=wh[:], in_=fwh[m])
                pss = [pspool.tile([128, 512], F32, name=f"ps{n}", tag="ps")
                       for n in range(NT)]
                for k in range(KT):
                    for n in range(NT):
                        nc.tensor.matmul(
                            pss[n][:],
                            wh[:, k],
                            xh_t[k][:, n * 512:(n + 1) * 512],
                            start=(k == 0),
                            stop=(k == KT - 1),
                        )
                for n in range(NT):
                    hf = hpool.tile([128, 512], BF16, name="hf", tag="hf")
                    nc.scalar.activation(
                        hf[:], pss[n][:], mybir.ActivationFunctionType.Relu,
                        bias=fb1_sb[:, m:m + 1],
                    )
                    nc.tensor.matmul(
                        ps2_f[n][:],
                        fw2_sb[:, 2 * m:2 * m + 2],
                        hf[:],
                        start=(m == 0),
                        stop=(m == MT - 1),
                    )
            for n in range(NT):
                lg = epool.tile([2, 512], F32, name=f"flg{n}")
                nc.vector.tensor_scalar_add(lg[:], ps2_f[n][:], fb2_sb[:, 0:1])
                logits[("f", n)] = lg

            # ---------------- routing epilogue ----------------
            for n in range(NT):
                tl, fl = logits[("t", n)], logits[("f", n)]
                # d = t0 - t1 via PE: [2,1]^T @ [2,512] -> [1,512]
                ps_d = ps2pool.tile([1, 512], F32, name=f"psd{n}", tag="psd")
                nc.tensor.matmul(ps_d[:], dvec_sb[:], tl[:], start=True, stop=True)
                ad = epool.tile([1, 512], F32, name=f"ad{n}")
                nc.scalar.activation(ad[:], ps_d[:], mybir.ActivationFunctionType.Abs)
                # broadcast |d| to both partitions: [1,2]^T @ [1,512] -> [2,512]
                ps_b = ps2pool.tile([2, 512], F32, name=f"psb{n}", tag="psd")
                nc.tensor.matmul(ps_b[:], ones_sb[:], ad[:], start=True, stop=True)
                mk = epool.tile([2, 512], mybir.dt.uint8, name=f"mk{n}")
                nc.vector.tensor_scalar(
                    mk[:], ps_b[:], LN4, None, op0=mybir.AluOpType.is_lt
                )
                ob = epool.tile([2, 512], F32, name=f"ob{n}")
                nc.vector.select(ob[:], mk[:], fl[:], tl[:])
                nc.sync.dma_start(out=out[:, n * 512:(n + 1) * 512], in_=ob[:])

    _prune_weight_dma_waits(nc)
    _fix_wait_overflow(nc)
    return nc


def _fix_wait_overflow(nc):
    """Walrus engine/DMA instructions accept at most 2 sync commands
    (waits + updates) total, but InstDrain accepts many waits. For any
    instruction exceeding the budget, hoist the extra waits onto an
    InstDrain inserted just before it on the same engine queue — the
    queue is in-order, so the following instruction only issues after
    the drain's waits are satisfied."""
    import concourse.mybir as _mybir

    seq = 0
    for bb in nc.m.functions[0].blocks:
        out_list = []
        for ins in bb.instructions:
            si = getattr(ins, "sync_info", None)
            if si is not None and type(ins).__name__ == "InstDrain":
                # split over-budget drains into a chain of <=2-wait drains
                waits = list(si.on_wait or [])
                if len(waits) + len(si.on_update or []) > 2:
                    while len(waits) > 1:
                        chunk, waits = waits[:1], waits[1:]
                        dr = _mybir.InstDrain(
                            name=f"WOF-{seq}", engine=ins.engine, ins=[], outs=[],
                            sync_info=_mybir.SyncInfo(on_wait=chunk, on_update=[]),
                        )
                        seq += 1
                        out_list.append(dr)
                    ins.sync_info = _mybir.SyncInfo(
                        on_wait=waits, on_update=si.on_update
                    )
                out_list.append(ins)
                continue
            if (
                si is not None
                and len(si.on_wait or []) + len(si.on_update or []) > 2
            ):
                n_upd = len(si.on_update or [])
                keep = max(0, 2 - n_upd - 1) + 1 if n_upd <= 1 else 0
                keep = min(keep, len(si.on_wait))
                extras = list(si.on_wait[keep:])
                if extras:
                    for i in range(0, len(extras), 1):
                        dr = _mybir.InstDrain(
                            name=f"WOF-{seq}",
                            engine=ins.engine,
                            ins=[],
                            outs=[],
                            sync_info=_mybir.SyncInfo(
                                on_wait=extras[i:i + 1], on_update=[]
                            ),
                        )
                        seq += 1
                        out_list.append(dr)
                    ins.sync_info = _mybir.SyncInfo(
                        on_wait=list(si.on_wait[:keep]), on_update=si.on_update
                    )
            out_list.append(ins)
        bb.instructions[:] = out_list


def _prune_weight_dma_waits(nc):
    """Walrus allows a single sem wait per DMA instruction, but Tile emits
    [PE-WAR, DMA-lane-WAW] pairs on the recycled weight-stream slots. The
    DMA-lane waits are redundant: the kept PE wait covers the tick of the
    last matmul that read the old slot contents, and that matmul itself
    waited on the old slot's DMA before reading. Drop the DMA-lane waits.

    Only applied to DMAs sourced from the weight-stream DRAM tensors, whose
    slots are written by DMA and read exclusively by the PE."""
    import concourse.mybir as _mybir

    wsrc = {"twh", "twl", "fwh"}
    for bb in nc.m.functions[0].blocks:
        for ins in bb.instructions:
            if type(ins).__name__ != "InstDMACopy":
                continue
            si = ins.sync_info
            if si is None or len(si.on_wait or []) <= 1:
                continue
            src = getattr(ins.ins[0], "memref", None)
            assert src in wsrc, f"unexpected multi-wait DMA from {src}"
            pe = [w for w in si.on_wait if w.ant_name.startswith("PE")]
            dropped = [w for w in si.on_wait if not w.ant_name.startswith("PE")]
            assert len(pe) == 1 and all(
                w.ant_name.startswith(("DMAHW", "DMASW")) for w in dropped
            ), f"unexpected wait mix on {ins.name}: " \
               f"{[w.ant_name for w in si.on_wait]}"
            ins.sync_info = _mybir.SyncInfo(on_wait=pe, on_update=si.on_update)


def _bf(a):
    return a.astype(ml_dtypes.bfloat16)


def _prep_w1(w, KT, MT):
    """[D,H] -> [MT, 128, KT*128] with w_pre[m,p,k*128+c] = w[k*128+p, m*128+c]"""
    D_, H_ = w.shape
    return np.ascontiguousarray(
        w.reshape(KT, 128, MT, 128).transpose(2, 1, 0, 3).reshape(MT, 128, KT * 128)
    )


def _prep_b1(b, MT):
    return np.ascontiguousarray(b.reshape(MT, 128).T)


def _prep_w2(w, MT):
    return np.ascontiguousarray(
        w.reshape(MT, 128, 2).transpose(1, 0, 2).reshape(128, MT * 2)
    )


_CACHED = {}


def _get_nc(Bc, D_, H_):
    key = (Bc, D_, H_)
    if key not in _CACHED:
        _CACHED[key] = build_nc(Bc, D_, H_)
    return _CACHED[key]


LAST_EXEC_TIME_NS = None


def kernel(x, t_w1, t_b1, t_w2, t_b2, f_w1, f_b1, f_w2, f_b2, _trace=False):
    global LAST_EXEC_TIME_NS
    x = np.asarray(x, dtype=np.float32)
    Bfull, D_ = x.shape
    H_ = t_w1.shape[1]
    Bc = Bfull // N_CORES
    KT, MT = D_ // 128, H_ // 128

    t_w1 = np.asarray(t_w1, dtype=np.float32)
    f_w1 = np.asarray(f_w1, dtype=np.float32)

    t_hi = _bf(t_w1)
    t_lo = _bf(t_w1 - t_hi.astype(np.float32))
    twh = _prep_w1(t_hi.astype(np.float32), KT, MT).astype(ml_dtypes.bfloat16)
    twl = _prep_w1(t_lo.astype(np.float32), KT, MT).astype(ml_dtypes.bfloat16)
    fwh = _prep_w1(_bf(f_w1).astype(np.float32), KT, MT).astype(ml_dtypes.bfloat16)

    tb1s = _prep_b1(np.asarray(t_b1, np.float32), MT)
    fb1s = _prep_b1(np.asarray(f_b1, np.float32), MT)
    tw2s = _prep_w2(np.asarray(t_w2, np.float32), MT)
    fw2s = _prep_w2(np.asarray(f_w2, np.float32), MT).astype(ml_dtypes.bfloat16)
    tb2s = np.asarray(t_b2, np.float32).reshape(2, 1)
    fb2s = np.asarray(f_b2, np.float32).reshape(2, 1)

    shared = dict(
        twh=twh, twl=twl, fwh=fwh, tb1=tb1s, fb1=fb1s,
        tw2=tw2s, fw2=fw2s, tb2=tb2s, fb2=fb2s,
        dvec=np.array([[1.0], [-1.0]], np.float32),
        ones12=np.array([[1.0, 1.0]], np.float32),
    )
    in_maps = []
    for c in range(N_CORES):
        xc = np.ascontiguousarray(x[c * Bc:(c + 1) * Bc].T)  # [D, Bc]
        xc_hi = _bf(xc)
        xc_lo = _bf(xc - xc_hi.astype(np.float32))
        in_maps.append(dict(shared, xhi=xc_hi, xlo=xc_lo))

    nc = _get_nc(Bc, D_, H_)
    res = run_bass_kernel_spmd(nc, in_maps, list(range(N_CORES)), trace=_trace)
    LAST_EXEC_TIME_NS = res.exec_time_ns
    outT = np.concatenate([res.results[c]["out"] for c in range(N_CORES)], axis=1)
    return np.ascontiguousarray(outT.T.astype(np.float32))

